# revision 12
# baseline (speedup 1.0000x reference)
"""DynamicA8W8Linear on 8 TRN2 NeuronCores.

Strategy: token-parallel (data-parallel) sharding. Each core processes
N/8 = 1024 tokens: per-token dynamic int8 quantization, int-exact bf16
matmul against the (replicated) int8 weight, fused dequant epilogue.

Layouts (host-prepared, all zero-FLOP reshapes/casts):
  x_t      [K, T]            fp32  per-core token shard, transposed
  w_t      [D/128,128,K/128,128] bf16  w[128d+n, 128i+p] -> w_t[d,p,i,n]
  smooth_t [128, K/128]      fp32  smooth[128i+p] -> [p, i]
  wscale_t [128, D/128]      fp32  wscale[128d+p] -> [p, d]
  bias_t   [128, D/128]      fp32
  out_t    [D, T]            fp32  transposed output (host re-transposes)

Numerics: xq values are integers in [-127,127] stored exactly in bf16;
products are exact in fp32 PSUM accumulation (|acc| << 2^24), so the
integer matmul matches the int8 reference exactly. Rounding uses the
(x + 1.5*2^23) - 1.5*2^23 trick = IEEE round-half-even = jnp.round.
"""

import sys

if "/opt/trn_rl_repo" not in sys.path:
    sys.path.insert(0, "/opt/trn_rl_repo")

import numpy as np
import ml_dtypes

import concourse.bass as bass
import concourse.bass_isa as bass_isa
import concourse.mybir as mybir
import concourse.tile as tile
from concourse import bacc
from concourse.bass_utils import run_bass_kernel_spmd

F32 = mybir.dt.float32
BF16 = mybir.dt.bfloat16
MUL = mybir.AluOpType.mult
ADD = mybir.AluOpType.add
SUB = mybir.AluOpType.subtract
MAX = mybir.AluOpType.max
DIV = mybir.AluOpType.divide

MAGIC = float(np.float32(12582912.0))  # 1.5 * 2**23, round-half-even trick


def build_program(T=1024, K=4096, D=4096, TC=256, TB=512, use_divide=False, repeat=1):
    """Emit the per-core Bass/Tile program. All 8 cores run this SPMD.

    repeat>1 wraps the whole body in a hardware For_i loop — used only for
    timing (per-iteration device time = (t(R)-t(1))/(R-1), RTT cancels).
    """
    KT = K // 128
    DT = D // 128
    n_chunks = T // TC
    n_blocks = T // TB

    nc = bacc.Bacc(None, target_bir_lowering=False)
    x_t = nc.dram_tensor("x_t", [K, T], F32, kind="ExternalInput")
    w_t = nc.dram_tensor("w_t", [DT, 128, KT, 128], BF16, kind="ExternalInput")
    smooth_t = nc.dram_tensor("smooth_t", [128, KT], F32, kind="ExternalInput")
    wscale_t = nc.dram_tensor("wscale_t", [128, DT], F32, kind="ExternalInput")
    bias_t = nc.dram_tensor("bias_t", [128, DT], F32, kind="ExternalInput")
    out_t = nc.dram_tensor("out_t", [D, T], F32, kind="ExternalOutput")
    sdram = nc.dram_tensor("s_scratch", [T], F32)
    rdram = nc.dram_tensor("r_scratch", [T], F32)

    x_v = x_t[:].rearrange("(i p) t -> p i t", p=128)

    with tile.TileContext(nc) as tc:
        with (
            tc.tile_pool(name="const", bufs=1) as const,
            tc.tile_pool(name="xq", bufs=1) as xqp,
            tc.tile_pool(name="xin", bufs=2) as xinp,
            tc.tile_pool(name="stat", bufs=2) as statp,
            tc.tile_pool(name="sbc", bufs=2) as sbcp,
            tc.tile_pool(name="wstrip", bufs=3) as wp,
            tc.tile_pool(name="outs", bufs=4) as outp,
            tc.tile_pool(name="psum", bufs=6, space="PSUM") as psp,
        ):
            smooth_sb = const.tile([128, KT], F32)
            nc.sync.dma_start(smooth_sb[:], smooth_t[:])
            wscale_sb = const.tile([128, DT], F32)
            nc.sync.dma_start(wscale_sb[:], wscale_t[:])
            bias_sb = const.tile([128, DT], F32)
            nc.sync.dma_start(bias_sb[:], bias_t[:])

            xq_sb = xqp.tile([128, KT, T], BF16)

            def emit_body():
                # ---- per-token dynamic quantization, chunks of TC tokens ----
                for c in range(n_chunks):
                    tsl = slice(c * TC, (c + 1) * TC)
                    xt = xinp.tile([128, KT, TC], F32)
                    nc.sync.dma_start(xt[:], x_v[:, :, tsl])

                    # xs = x * smooth, in place
                    smooth_bc = smooth_sb[:].unsqueeze(2).broadcast_to([128, KT, TC])
                    nc.vector.tensor_tensor(xt[:], xt[:], smooth_bc, MUL)

                    # per-token absmax: reduce over i (innermost after swap) ...
                    macc = statp.tile([128, TC], F32)
                    nc.vector.tensor_reduce(
                        macc[:],
                        xt[:].rearrange("p i t -> p t i"),
                        axis=mybir.AxisListType.X,
                        op=MAX,
                        apply_absolute_value=True,
                    )
                    # ... then across partitions (result lands on all partitions)
                    m_all = statp.tile([128, TC], F32)
                    nc.gpsimd.partition_all_reduce(
                        m_all[:], macc[:], 128, bass_isa.ReduceOp.max
                    )

                    # s = m/127 (in place), stash row for epilogue, r = 1/s (in place)
                    nc.vector.tensor_scalar(
                        m_all[:], m_all[:], float(np.float32(1.0 / 127.0)), None, MUL
                    )
                    nc.sync.dma_start(sdram[tsl].unsqueeze(0), m_all[0:1, :])
                    nc.vector.reciprocal(m_all[:], m_all[:])

                    # xq = round_half_even(xs * r) as exact bf16 integers
                    nc.vector.tensor_tensor(
                        xt[:], xt[:], m_all[:].unsqueeze(1).broadcast_to([128, KT, TC]), MUL
                    )
                    nc.vector.tensor_scalar(
                        xq_sb[:, :, tsl], xt[:], MAGIC, MAGIC, ADD, SUB
                    )

                # ---- matmul + fused dequant, token blocks of TB ----
                for b in range(n_blocks):
                    bsl = slice(b * TB, (b + 1) * TB)
                    sbc = sbcp.tile([128, TB], F32)
                    nc.sync.dma_start(sbc[:], sdram[bsl].unsqueeze(0).broadcast_to([128, TB]))
                    for d in range(DT):
                        wsb = wp.tile([128, KT, 128], BF16)
                        nc.sync.dma_start(wsb[:], w_t[d])
                        ps = psp.tile([128, TB], F32)
                        for i in range(KT):
                            nc.tensor.matmul(
                                ps[:],
                                wsb[:, i, :],
                                xq_sb[:, i, bsl],
                                start=(i == 0),
                                stop=(i == KT - 1),
                            )
                        ot = outp.tile([128, TB], F32)
                        # out = (acc * wscale[p]) * x_scale[t]  ... + bias[p]
                        nc.vector.scalar_tensor_tensor(
                            ot[:], ps[:], wscale_sb[:, d : d + 1], sbc[:], MUL, MUL
                        )
                        nc.vector.tensor_scalar(
                            ot[:], ot[:], bias_sb[:, d : d + 1], None, ADD
                        )
                        nc.sync.dma_start(out_t[d * 128 : (d + 1) * 128, bsl], ot[:])

            if repeat > 1:
                with tc.For_i(0, repeat, 1):
                    emit_body()
            else:
                emit_body()

    nc.compile()
    return nc


def host_pack(x2d_shard, weight, smooth_scales, weight_scale, bias):
    """Zero-FLOP host-side layout prep for one core."""
    K = x2d_shard.shape[1]
    D = weight.shape[0]
    KT, DT = K // 128, D // 128
    x_tp = np.ascontiguousarray(x2d_shard.T)
    w4 = np.ascontiguousarray(
        weight.reshape(DT, 128, KT, 128).transpose(0, 3, 2, 1)
    ).astype(ml_dtypes.bfloat16)
    st = np.ascontiguousarray(smooth_scales.reshape(KT, 128).T)
    wst = np.ascontiguousarray(weight_scale.reshape(DT, 128).T)
    bt = np.ascontiguousarray(bias.reshape(DT, 128).T)
    return {
        "x_t": x_tp,
        "w_t": w4,
        "smooth_t": st,
        "wscale_t": wst,
        "bias_t": bt,
    }


_PROGRAM_CACHE = {}


def kernel(x, weight, smooth_scales, weight_scale, bias):
    x = np.asarray(x)
    weight = np.asarray(weight)
    smooth_scales = np.asarray(smooth_scales, dtype=np.float32)
    weight_scale = np.asarray(weight_scale, dtype=np.float32)
    bias = np.asarray(bias, dtype=np.float32)

    out_shape = x.shape[:-1] + (weight.shape[0],)
    K = x.shape[-1]
    D = weight.shape[0]
    x2d = np.ascontiguousarray(x.reshape(-1, K), dtype=np.float32)
    N = x2d.shape[0]
    n_cores = 8
    T = N // n_cores

    key = (T, K, D)
    if key not in _PROGRAM_CACHE:
        _PROGRAM_CACHE[key] = build_program(T=T, K=K, D=D)
    nc = _PROGRAM_CACHE[key]

    # weight-derived inputs are identical on every core; pack once
    shared = host_pack(x2d[:T], weight, smooth_scales, weight_scale, bias)
    in_maps = []
    for c in range(n_cores):
        m = dict(shared)
        m["x_t"] = np.ascontiguousarray(x2d[c * T : (c + 1) * T].T)
        in_maps.append(m)

    res = run_bass_kernel_spmd(nc, in_maps, list(range(n_cores))).results

    out2d = np.empty((N, D), dtype=np.float32)
    for c in range(n_cores):
        out2d[c * T : (c + 1) * T] = res[c]["out_t"].T
    return out2d.reshape(out_shape)


# revision 16
# speedup vs baseline: 1.0914x; 1.0914x over previous
"""DynamicA8W8Linear on 8 TRN2 NeuronCores.

Strategy: token-parallel (data-parallel) sharding. Each core processes
N/8 = 1024 tokens: per-token dynamic int8 quantization, int-exact bf16
matmul against the (replicated) int8 weight, fused dequant epilogue.

Layouts (host-prepared, all zero-FLOP reshapes/casts):
  x_t      [K, T]            fp32  per-core token shard, transposed
  w_t      [D/128,128,K/128,128] bf16  w[128d+n, 128i+p] -> w_t[d,p,i,n]
  smooth_t [128, K/128]      fp32  smooth[128i+p] -> [p, i]
  wscale_t [128, D/128]      fp32  wscale[128d+p] -> [p, d]
  bias_t   [128, D/128]      fp32
  out_t    [D, T]            fp32  transposed output (host re-transposes)

Numerics: xq values are integers in [-127,127] stored exactly in bf16;
products are exact in fp32 PSUM accumulation (|acc| << 2^24), so the
integer matmul matches the int8 reference exactly. Rounding uses the
(x + 1.5*2^23) - 1.5*2^23 trick = IEEE round-half-even = jnp.round.
"""

import sys

if "/opt/trn_rl_repo" not in sys.path:
    sys.path.insert(0, "/opt/trn_rl_repo")

import numpy as np
import ml_dtypes

import concourse.bass as bass
import concourse.bass_isa as bass_isa
import concourse.mybir as mybir
import concourse.tile as tile
from concourse import bacc
from concourse.bass_utils import run_bass_kernel_spmd

F32 = mybir.dt.float32
BF16 = mybir.dt.bfloat16
MUL = mybir.AluOpType.mult
ADD = mybir.AluOpType.add
SUB = mybir.AluOpType.subtract
MAX = mybir.AluOpType.max
DIV = mybir.AluOpType.divide

MAGIC = float(np.float32(12582912.0))  # 1.5 * 2**23, round-half-even trick


def build_program(
    T=1024, K=4096, D=4096, TC=256, TB=512, use_divide=False, repeat=1,
    no_gpsimd=False, psum_bufs=6, w_bufs=3, xin_bufs=2, out_bufs=4,
):
    """Emit the per-core Bass/Tile program. All 8 cores run this SPMD.

    repeat>1 wraps the whole body in a hardware For_i loop — used only for
    timing (per-iteration device time = (t(R)-t(1))/(R-1), RTT cancels).
    """
    KT = K // 128
    DT = D // 128
    n_chunks = T // TC
    n_blocks = T // TB

    nc = bacc.Bacc(None, target_bir_lowering=False)
    x_t = nc.dram_tensor("x_t", [K, T], F32, kind="ExternalInput")
    w_t = nc.dram_tensor("w_t", [DT, 128, KT, 128], BF16, kind="ExternalInput")
    smooth_t = nc.dram_tensor("smooth_t", [128, KT], F32, kind="ExternalInput")
    wscale_t = nc.dram_tensor("wscale_t", [128, DT], F32, kind="ExternalInput")
    bias_t = nc.dram_tensor("bias_t", [128, DT], F32, kind="ExternalInput")
    out_t = nc.dram_tensor("out_t", [D, T], F32, kind="ExternalOutput")
    sdram = nc.dram_tensor("s_scratch", [T], F32)
    rdram = nc.dram_tensor("r_scratch", [T], F32)

    x_v = x_t[:].rearrange("(i p) t -> p i t", p=128)

    with tile.TileContext(nc) as tc:
        with (
            tc.tile_pool(name="const", bufs=1) as const,
            tc.tile_pool(name="xq", bufs=1) as xqp,
            tc.tile_pool(name="xin", bufs=xin_bufs) as xinp,
            tc.tile_pool(name="stat", bufs=2) as statp,
            tc.tile_pool(name="sbc", bufs=2) as sbcp,
            tc.tile_pool(name="wstrip", bufs=w_bufs) as wp,
            tc.tile_pool(name="outs", bufs=out_bufs) as outp,
            tc.tile_pool(name="psum", bufs=psum_bufs, space="PSUM") as psp,
        ):
            smooth_sb = const.tile([128, KT], F32)
            nc.sync.dma_start(smooth_sb[:], smooth_t[:])
            wscale_sb = const.tile([128, DT], F32)
            nc.sync.dma_start(wscale_sb[:], wscale_t[:])
            bias_sb = const.tile([128, DT], F32)
            nc.sync.dma_start(bias_sb[:], bias_t[:])

            # one xq tile per token block so block-b matmuls depend only
            # on that block's quant chunks (fine-grained overlap)
            xq_blocks = [
                xqp.tile([128, KT, TB], BF16, tag=f"xq{b}", name=f"xq{b}")
                for b in range(n_blocks)
            ]
            cpb = TB // TC  # quant chunks per block

            def emit_quant_chunk(c):
                b = c // cpb
                xq_sb = xq_blocks[b]
                qsl = slice((c % cpb) * TC, (c % cpb + 1) * TC)
                tsl = slice(c * TC, (c + 1) * TC)
                xt = xinp.tile([128, KT, TC], F32, tag="xt", name="xt")
                nc.scalar.dma_start(xt[:], x_v[:, :, tsl])

                # xs = x * smooth, in place
                smooth_bc = smooth_sb[:].unsqueeze(2).broadcast_to([128, KT, TC])
                nc.vector.tensor_tensor(xt[:], xt[:], smooth_bc, MUL)

                # per-token absmax: reduce over i (innermost after swap) ...
                macc = statp.tile([128, TC], F32, tag="macc", name="macc")
                nc.vector.tensor_reduce(
                    macc[:],
                    xt[:].rearrange("p i t -> p t i"),
                    axis=mybir.AxisListType.X,
                    op=MAX,
                    apply_absolute_value=True,
                )
                # ... then across partitions (result lands on all partitions)
                m_all = statp.tile([128, TC], F32, tag="m_all", name="m_all")
                if no_gpsimd:
                    # timing probe only: wrong numerics, no gpsimd port lock
                    nc.vector.tensor_copy(m_all[:], macc[:])
                else:
                    nc.gpsimd.partition_all_reduce(
                        m_all[:], macc[:], 128, bass_isa.ReduceOp.max
                    )

                # s = m/127 (in place), stash row for epilogue, r = 1/s (in place)
                nc.vector.tensor_scalar(
                    m_all[:], m_all[:], float(np.float32(1.0 / 127.0)), None, MUL
                )
                nc.scalar.dma_start(sdram[tsl].unsqueeze(0), m_all[0:1, :])
                nc.vector.reciprocal(m_all[:], m_all[:])

                # xq = round_half_even(xs * r) as exact bf16 integers
                nc.vector.tensor_tensor(
                    xt[:], xt[:], m_all[:].unsqueeze(1).broadcast_to([128, KT, TC]), MUL
                )
                nc.vector.tensor_scalar(
                    xq_sb[:, :, qsl], xt[:], MAGIC, MAGIC, ADD, SUB
                )

            def emit_matmul_block(b):
                bsl = slice(b * TB, (b + 1) * TB)
                xq_sb = xq_blocks[b]
                sbc = sbcp.tile([128, TB], F32, tag="sbc", name="sbc")
                nc.scalar.dma_start(
                    sbc[:], sdram[bsl].unsqueeze(0).broadcast_to([128, TB])
                )
                for d in range(DT):
                    wsb = wp.tile([128, KT, 128], BF16, tag="wsb", name="wsb")
                    nc.sync.dma_start(wsb[:], w_t[d])
                    ps = psp.tile([128, TB], F32, tag="ps", name="ps")
                    for i in range(KT):
                        nc.tensor.matmul(
                            ps[:],
                            wsb[:, i, :],
                            xq_sb[:, i, :],
                            start=(i == 0),
                            stop=(i == KT - 1),
                        )
                    ot = outp.tile([128, TB], F32, tag="ot", name="ot")
                    # out = (acc * wscale[p]) * x_scale[t]  ... + bias[p]
                    nc.vector.scalar_tensor_tensor(
                        ot[:], ps[:], wscale_sb[:, d : d + 1], sbc[:], MUL, MUL
                    )
                    nc.vector.tensor_scalar(
                        ot[:], ot[:], bias_sb[:, d : d + 1], None, ADD
                    )
                    nc.scalar.dma_start(out_t[d * 128 : (d + 1) * 128, bsl], ot[:])

            def emit_body():
                for b in range(n_blocks):
                    for c in range(b * cpb, (b + 1) * cpb):
                        emit_quant_chunk(c)
                    emit_matmul_block(b)

            if repeat > 1:
                with tc.For_i(0, repeat, 1):
                    emit_body()
            else:
                emit_body()

    nc.compile()
    return nc


def host_pack(x2d_shard, weight, smooth_scales, weight_scale, bias):
    """Zero-FLOP host-side layout prep for one core."""
    K = x2d_shard.shape[1]
    D = weight.shape[0]
    KT, DT = K // 128, D // 128
    x_tp = np.ascontiguousarray(x2d_shard.T)
    w4 = np.ascontiguousarray(
        weight.reshape(DT, 128, KT, 128).transpose(0, 3, 2, 1)
    ).astype(ml_dtypes.bfloat16)
    st = np.ascontiguousarray(smooth_scales.reshape(KT, 128).T)
    wst = np.ascontiguousarray(weight_scale.reshape(DT, 128).T)
    bt = np.ascontiguousarray(bias.reshape(DT, 128).T)
    return {
        "x_t": x_tp,
        "w_t": w4,
        "smooth_t": st,
        "wscale_t": wst,
        "bias_t": bt,
    }


_PROGRAM_CACHE = {}


def kernel(x, weight, smooth_scales, weight_scale, bias):
    x = np.asarray(x)
    weight = np.asarray(weight)
    smooth_scales = np.asarray(smooth_scales, dtype=np.float32)
    weight_scale = np.asarray(weight_scale, dtype=np.float32)
    bias = np.asarray(bias, dtype=np.float32)

    out_shape = x.shape[:-1] + (weight.shape[0],)
    K = x.shape[-1]
    D = weight.shape[0]
    x2d = np.ascontiguousarray(x.reshape(-1, K), dtype=np.float32)
    N = x2d.shape[0]
    n_cores = 8
    T = N // n_cores

    key = (T, K, D)
    if key not in _PROGRAM_CACHE:
        _PROGRAM_CACHE[key] = build_program(T=T, K=K, D=D)
    nc = _PROGRAM_CACHE[key]

    # weight-derived inputs are identical on every core; pack once
    shared = host_pack(x2d[:T], weight, smooth_scales, weight_scale, bias)
    in_maps = []
    for c in range(n_cores):
        m = dict(shared)
        m["x_t"] = np.ascontiguousarray(x2d[c * T : (c + 1) * T].T)
        in_maps.append(m)

    res = run_bass_kernel_spmd(nc, in_maps, list(range(n_cores))).results

    out2d = np.empty((N, D), dtype=np.float32)
    for c in range(n_cores):
        out2d[c * T : (c + 1) * T] = res[c]["out_t"].T
    return out2d.reshape(out_shape)
